# revision 1
# baseline (speedup 1.0000x reference)
"""DAG-constraint layer kernel for Trainium2 (8 NeuronCores, data parallel).

The reference computes p = sigmoid(x) followed by an iterative min/max
projection over a fixed chain+skip DAG on N=32 nodes (children of i are
{i+1, i+2}).  On that DAG the projection's fixed point is reached after a
single iteration and collapses to the prefix-min along the node axis:

    out[b, j] = min_{k <= j} sigmoid(x[b, k]) = sigmoid(cummin(x, axis=1))

(verified bitwise against the reference).  So the kernel is a per-row
prefix-min over 32 columns plus a sigmoid - purely memory bound.

Per core: rows are sharded 8 ways (65536 rows x 32 f32 = 8 MiB per shard).
The shard is processed as [128 partitions x F free] tiles; each partition
holds F/32 complete rows, so each row's 32 columns are contiguous in the
free dimension.  The prefix-min of many rows is computed with one hardware
scan instruction (TensorTensorScanArith) per tile:

    state_t = max( min(x_t, state_{t-1}), C_t )

where C is a constant: +BIG at each row's LAST column (t % 32 == 31) and
-BIG elsewhere.  The +BIG poisons the state at each row end, so the next
row starts a fresh running min (initial=+BIG handles the first row).  Each
row's column 31 then holds +BIG instead of the true value; one cheap
strided min (64 elements/partition) repairs it:
    q[:, 31::32] = min(q[:, 30::32], x[:, 31::32])
Sigmoid runs on the scalar engine in place.

Raw bass (explicit semaphores) rather than Tile: the walrus build in this
container only encodes a single sync-wait per instruction, so waits are
issued as standalone wait_ge commands.  Pipeline: sync engine issues input
DMAs (plus a gated SWDGE prefetch of the tail tiles on gpsimd, a third DMA
ring), vector (DVE) runs scan+fix, scalar (ACT) runs sigmoid and issues
output DMAs.  Per-tile input semaphores give exact completion; the single
output semaphore is only waited at its total.

kernel() runs in-process when the 8 NeuronCores are visible to jax;
otherwise (e.g. the caller pinned jax to CPU) it re-executes itself in a
clean subprocess.
"""

import os
import subprocess
import sys
import tempfile
from contextlib import ExitStack

import numpy as np

import concourse.bass as bass
import concourse.mybir as mybir
from concourse.bass_utils import run_bass_kernel_spmd

N_CORES = 8
B_TOTAL = 524288
N_NODES = 32
ROWS_PER_CORE = B_TOTAL // N_CORES  # 65536
P = 128                             # SBUF partitions
# Per-tile free-dim sizes (f32 elements per partition).  Small tiles at the
# head shorten the pipeline fill (first scan can start ~3us earlier);
# moderate tiles at the tail shorten the drain (last scan->sigmoid->store
# chain) while staying >= 1024 so their column-31 fix can run immediately
# after the scan (see the hazard note in the vector block).
FSIZES = [512, 512, 1024] + [2048] * 6 + [1024, 1024]
FMAX = max(FSIZES)
NT = len(FSIZES)
NEG_BIG = -3.0e38
POS_BIG = 3.0e38

assert sum(FSIZES) * P == ROWS_PER_CORE * N_NODES
assert all(f % N_NODES == 0 for f in FSIZES)


def _col(ap, c):
    """AP selecting column c of every N_NODES-wide row: [P, F/N] stride N."""
    return ap[:].rearrange("p (g n) -> p g n", n=N_NODES)[:, :, c]


def _build() -> bass.Bass:
    nc = bass.Bass()
    f32 = mybir.dt.float32
    x = nc.declare_dram_parameter("x", [ROWS_PER_CORE, N_NODES], f32, isOutput=False)
    y = nc.declare_dram_parameter("y", [ROWS_PER_CORE, N_NODES], f32, isOutput=True)
    xf = x[:].flatten()
    yf = y[:].flatten()
    # DRAM chunk per tile t: contiguous [P, FSIZES[t]] starting at offset[t]
    offs = [0]
    for fsz in FSIZES:
        offs.append(offs[-1] + P * fsz)

    def _dram_tile(flat, t):
        return flat[offs[t] : offs[t + 1]].rearrange("(p f) -> p f", p=P)

    with ExitStack() as es:
        ec = es.enter_context
        # All NT tiles resident at once (17 MiB of SBUF): no slot reuse, so
        # the input DMA stream runs with no dependency on compute at all.
        xts = [ec(nc.sbuf_tensor(f"xt{i}", [P, FSIZES[i]], f32)) for i in range(NT)]
        qts = [ec(nc.sbuf_tensor(f"qt{i}", [P, FSIZES[i]], f32)) for i in range(NT)]
        cmask = ec(nc.sbuf_tensor("cmask", [P, FMAX], f32))
        warm = ec(nc.sbuf_tensor("act_warm", [P, 1], f32))
        sep = ec(nc.sbuf_tensor("sep", [P, 64], f32))
        # Per-tile input semaphores: a cumulative count over several
        # in-flight DMAs is NOT a completion indicator (the 16 per-SDMA-
        # engine increments of different DMAs interleave), but with one DMA
        # per semaphore the count is exact.  The single output semaphore is
        # only ever waited at its total (all increments fired), so a shared
        # counter is fine there.
        dma_in = [ec(nc.semaphore(f"dma_in{i}")) for i in range(NT)]
        dma_out = ec(nc.semaphore("dma_out"))
        scan_sem = ec(nc.semaphore("scan_sem"))
        gp_sem = ec(nc.semaphore("gp_sem"))
        act_sem = ec(nc.semaphore("act_sem"))

        with nc.Block() as block:

            # The scan consumes input at ~246 GB/s while the shared SP ring
            # delivers ~236 GB/s mid-kernel - the tail tiles would arrive
            # just too late.  Ship the last two tiles through the separate
            # SWDGE (gpsimd) ring up front so they are resident early.
            SWDGE_TILES = {NT - 2, NT - 1}

            @block.sync
            def _(sync):
                for t in range(NT):
                    if t in SWDGE_TILES:
                        continue
                    sync.dma_start(
                        out=xts[t][:], in_=_dram_tile(xf, t)
                    ).then_inc(dma_in[t], 16)

            @block.gpsimd
            def _(gp):
                # Wait until the head tiles are through before adding SWDGE
                # traffic - early ring contention delays the pipeline start.
                gp.wait_ge(gp_sem, 3)
                for t in sorted(SWDGE_TILES):
                    gp.dma_start(
                        out=xts[t][:], in_=_dram_tile(xf, t)
                    ).then_inc(dma_in[t], 16)

            @block.vector
            def _(vector):
                def fix(t):
                    # Column-31 poison repair (walrus rejects tensor ops on
                    # GpSimd, so this stays on the vector engine).
                    vector.tensor_tensor(
                        out=_col(qts[t], N_NODES - 1),
                        in0=_col(qts[t], N_NODES - 2),
                        in1=_col(xts[t], N_NODES - 1),
                        op=mybir.AluOpType.min,
                    ).then_inc(gp_sem, 1)

                vector.memset(cmask[:], NEG_BIG)
                vector.memset(_col(cmask, N_NODES - 1), POS_BIG)
                # Hazard: the fix reads the scan's freshly written tail;
                # run back-to-back after a SHORT (F=512) scan the strided
                # read samples stale SBUF.  Empirically immediate fixes are
                # clean for F >= 1024; defer only the short head tiles' fixes
                # by one scan.  gp_sem increments stay in tile order.
                pending = None
                for t in range(NT):
                    vector.wait_ge(dma_in[t], 16)
                    vector.tensor_tensor_scan(
                        out=qts[t][:],
                        data0=xts[t][:],
                        data1=cmask[:, : FSIZES[t]],
                        initial=POS_BIG,
                        op0=mybir.AluOpType.min,
                        op1=mybir.AluOpType.max,
                    )
                    if pending is not None:
                        fix(pending)
                        pending = None
                    if FSIZES[t] >= 1024:
                        fix(t)
                    else:
                        pending = t
                if pending is not None:
                    vector.tensor_copy(out=sep[:], in_=cmask[:, :64])
                    fix(pending)

            @block.scalar
            def _(scalar):
                # Dummy activation: pulls the sigmoid table load (~2.7us)
                # off the first tile's critical path.  Contents are unused,
                # so the uninitialized tile is fine.
                scalar.activation(
                    out=warm[:], in_=warm[:],
                    func=mybir.ActivationFunctionType.Sigmoid,
                )
                for t in range(NT):
                    scalar.wait_ge(gp_sem, t + 1)
                    scalar.activation(
                        out=qts[t][:],
                        in_=qts[t][:],
                        func=mybir.ActivationFunctionType.Sigmoid,
                    ).then_inc(act_sem, 1)
                    # The sequencer dispatches the DMA before the ACTIVATE's
                    # writes land; gate on its completion explicitly.
                    scalar.wait_ge(act_sem, t + 1)
                    scalar.dma_start(
                        out=_dram_tile(yf, t), in_=qts[t][:]
                    ).then_inc(dma_out, 16)
                scalar.wait_ge(dma_out, 16 * NT)

    return nc


def _run(x: np.ndarray, trace: bool = False):
    x = np.ascontiguousarray(np.asarray(x), dtype=np.float32)
    assert x.shape == (B_TOTAL, N_NODES), x.shape
    nc = _build()
    in_maps = [
        {"x": x[i * ROWS_PER_CORE : (i + 1) * ROWS_PER_CORE]} for i in range(N_CORES)
    ]
    res = run_bass_kernel_spmd(nc, in_maps, list(range(N_CORES)), trace=trace)
    out = np.concatenate([res.results[i]["y"] for i in range(N_CORES)], axis=0)
    return out, res


def _trn_devices_visible() -> bool:
    """True when this process' jax backend exposes the 8 NeuronCores.
    A caller that pinned jax to CPU (e.g. to run the reference) hides them;
    in that case the bass run must happen in a clean subprocess."""
    try:
        import jax

        return sum(1 for d in jax.devices() if d.platform != "cpu") >= N_CORES
    except Exception:
        return False


def _run_in_subprocess(x: np.ndarray) -> np.ndarray:
    with tempfile.TemporaryDirectory() as td:
        xin = os.path.join(td, "x.npy")
        xout = os.path.join(td, "y.npy")
        np.save(xin, x)
        env = dict(os.environ)
        for k in ("JAX_PLATFORMS", "JAX_PLATFORM_NAME"):
            env.pop(k, None)
        subprocess.run(
            [sys.executable, os.path.abspath(__file__), xin, xout],
            check=True,
            env=env,
        )
        return np.load(xout)


def kernel(x, children=None, child_mask=None, parents=None, parent_mask=None,
           topo=None, **_unused):
    x = np.ascontiguousarray(np.asarray(x), dtype=np.float32)
    if _trn_devices_visible():
        out, _ = _run(x)
        return out
    return _run_in_subprocess(x)


if __name__ == "__main__":
    _x = np.load(sys.argv[1])
    _out, _ = _run(_x)
    np.save(sys.argv[2], _out)



# revision 5
# speedup vs baseline: 1.0501x; 1.0501x over previous
"""DAG-constraint layer kernel for Trainium2 (8 NeuronCores, data parallel).

The reference computes p = sigmoid(x) followed by an iterative min/max
projection over a fixed chain+skip DAG on N=32 nodes (children of i are
{i+1, i+2}).  On that DAG the projection's fixed point is reached after a
single iteration and collapses to the prefix-min along the node axis:

    out[b, j] = min_{k <= j} sigmoid(x[b, k]) = sigmoid(cummin(x, axis=1))

(verified bitwise against the reference).  So the kernel is a per-row
prefix-min over 32 columns plus a sigmoid - purely memory bound.

Per core: rows are sharded 8 ways (65536 rows x 32 f32 = 8 MiB per shard).
The shard is processed as [128 partitions x F free] tiles; each partition
holds F/32 complete rows, so each row's 32 columns are contiguous in the
free dimension.  The prefix-min of many rows is computed with one hardware
scan instruction (TensorTensorScanArith) per tile:

    state_t = max( min(x_t, state_{t-1}), C_t )

where C is a constant: +BIG at each row's LAST column (t % 32 == 31) and
-BIG elsewhere.  The +BIG poisons the state at each row end, so the next
row starts a fresh running min (initial=+BIG handles the first row).  Each
row's column 31 then holds +BIG instead of the true value; one cheap
strided min (64 elements/partition) repairs it:
    q[:, 31::32] = min(q[:, 30::32], x[:, 31::32])
Sigmoid runs on the scalar engine in place.

Raw bass (explicit semaphores) rather than Tile: the walrus build in this
container only encodes a single sync-wait per instruction, so waits are
issued as standalone wait_ge commands.  Pipeline: sync engine issues input
DMAs (plus a gated SWDGE prefetch of the tail tiles on gpsimd, a third DMA
ring), vector (DVE) runs scan+fix, scalar (ACT) runs sigmoid and issues
output DMAs.  Per-tile input semaphores give exact completion; the single
output semaphore is only waited at its total.

kernel() runs in-process when the 8 NeuronCores are visible to jax;
otherwise (e.g. the caller pinned jax to CPU) it re-executes itself in a
clean subprocess.
"""

import os
import subprocess
import sys
import tempfile
from contextlib import ExitStack

import numpy as np

import concourse.bass as bass
import concourse.mybir as mybir
from concourse.bass_utils import run_bass_kernel_spmd

N_CORES = 8
B_TOTAL = 524288
N_NODES = 32
ROWS_PER_CORE = B_TOTAL // N_CORES  # 65536
P = 128                             # SBUF partitions
# Per-tile free-dim sizes (f32 elements per partition).  Small tiles at the
# head shorten the pipeline fill (first scan can start ~3us earlier);
# moderate tiles at the tail shorten the drain (last scan->sigmoid->store
# chain) while staying >= 1024 so their column-31 fix can run immediately
# after the scan (see the hazard note in the vector block).
FSIZES = [512, 512, 1024] + [2048] * 6 + [1024, 1024]
FMAX = max(FSIZES)
NT = len(FSIZES)
# fp16 pipeline: tolerance is 2e-2 and fp16 end-to-end lands ~3e-3, so all
# SBUF tiles and both DRAM streams are float16 (half the HBM traffic).
# Poison values must be fp16-representable (fp16 max = 65504).
NEG_BIG = -60000.0
POS_BIG = 60000.0

assert sum(FSIZES) * P == ROWS_PER_CORE * N_NODES
assert all(f % N_NODES == 0 for f in FSIZES)


def _col(ap, c):
    """AP selecting column c of every N_NODES-wide row: [P, F/N] stride N."""
    return ap[:].rearrange("p (g n) -> p g n", n=N_NODES)[:, :, c]


def _build() -> bass.Bass:
    nc = bass.Bass()
    f16 = mybir.dt.float16
    x = nc.declare_dram_parameter("x", [ROWS_PER_CORE, N_NODES], f16, isOutput=False)
    y = nc.declare_dram_parameter("y", [ROWS_PER_CORE, N_NODES], f16, isOutput=True)
    xf = x[:].flatten()
    yf = y[:].flatten()
    # DRAM chunk per tile t: contiguous [P, FSIZES[t]] starting at offset[t]
    offs = [0]
    for fsz in FSIZES:
        offs.append(offs[-1] + P * fsz)

    def _dram_tile(flat, t):
        return flat[offs[t] : offs[t + 1]].rearrange("(p f) -> p f", p=P)

    with ExitStack() as es:
        ec = es.enter_context
        # All NT tiles resident at once (17 MiB of SBUF): no slot reuse, so
        # the input DMA stream runs with no dependency on compute at all.
        xts = [ec(nc.sbuf_tensor(f"xt{i}", [P, FSIZES[i]], f16)) for i in range(NT)]
        qts = [ec(nc.sbuf_tensor(f"qt{i}", [P, FSIZES[i]], f16)) for i in range(NT)]
        cmask = ec(nc.sbuf_tensor("cmask", [P, FMAX], f16))
        warm = ec(nc.sbuf_tensor("act_warm", [P, 1], f16))
        sep = ec(nc.sbuf_tensor("sep", [P, 64], f16))
        # Per-tile input semaphores: a cumulative count over several
        # in-flight DMAs is NOT a completion indicator (the 16 per-SDMA-
        # engine increments of different DMAs interleave), but with one DMA
        # per semaphore the count is exact.  The single output semaphore is
        # only ever waited at its total (all increments fired), so a shared
        # counter is fine there.
        dma_in = [ec(nc.semaphore(f"dma_in{i}")) for i in range(NT)]
        dma_out = ec(nc.semaphore("dma_out"))
        scan_sem = ec(nc.semaphore("scan_sem"))
        gp_sem = ec(nc.semaphore("gp_sem"))
        act_sem = ec(nc.semaphore("act_sem"))

        with nc.Block() as block:

            # The scan consumes input at ~246 GB/s while the shared SP ring
            # delivers ~236 GB/s mid-kernel - the tail tiles would arrive
            # just too late.  Ship the last two tiles through the separate
            # SWDGE (gpsimd) ring up front so they are resident early.
            SWDGE_TILES = {NT - 2, NT - 1}

            @block.sync
            def _(sync):
                for t in range(NT):
                    if t in SWDGE_TILES:
                        continue
                    sync.dma_start(
                        out=xts[t][:], in_=_dram_tile(xf, t)
                    ).then_inc(dma_in[t], 16)

            @block.gpsimd
            def _(gp):
                # Wait until the head tiles are through before adding SWDGE
                # traffic - early ring contention delays the pipeline start.
                gp.wait_ge(gp_sem, 3)
                for t in sorted(SWDGE_TILES):
                    gp.dma_start(
                        out=xts[t][:], in_=_dram_tile(xf, t)
                    ).then_inc(dma_in[t], 16)

            @block.vector
            def _(vector):
                def fix(t):
                    # Column-31 poison repair (walrus rejects tensor ops on
                    # GpSimd, so this stays on the vector engine).
                    vector.tensor_tensor(
                        out=_col(qts[t], N_NODES - 1),
                        in0=_col(qts[t], N_NODES - 2),
                        in1=_col(xts[t], N_NODES - 1),
                        op=mybir.AluOpType.min,
                    ).then_inc(gp_sem, 1)

                vector.memset(cmask[:], NEG_BIG)
                vector.memset(_col(cmask, N_NODES - 1), POS_BIG)
                # Hazard: the fix reads the scan's freshly written tail;
                # run back-to-back after a SHORT (F=512) scan the strided
                # read samples stale SBUF.  Empirically immediate fixes are
                # clean for F >= 1024; defer only the short head tiles' fixes
                # by one scan.  gp_sem increments stay in tile order.
                pending = None
                for t in range(NT):
                    vector.wait_ge(dma_in[t], 16)
                    vector.tensor_tensor_scan(
                        out=qts[t][:],
                        data0=xts[t][:],
                        data1=cmask[:, : FSIZES[t]],
                        initial=POS_BIG,
                        op0=mybir.AluOpType.min,
                        op1=mybir.AluOpType.max,
                    )
                    if pending is not None:
                        fix(pending)
                        pending = None
                    if FSIZES[t] >= 1024:
                        fix(t)
                    else:
                        pending = t
                if pending is not None:
                    vector.tensor_copy(out=sep[:], in_=cmask[:, :64])
                    fix(pending)

            @block.scalar
            def _(scalar):
                # Dummy activation: pulls the sigmoid table load (~2.7us)
                # off the first tile's critical path.  Contents are unused,
                # so the uninitialized tile is fine.
                scalar.activation(
                    out=warm[:], in_=warm[:],
                    func=mybir.ActivationFunctionType.Sigmoid,
                )
                for t in range(NT):
                    scalar.wait_ge(gp_sem, t + 1)
                    scalar.activation(
                        out=qts[t][:],
                        in_=qts[t][:],
                        func=mybir.ActivationFunctionType.Sigmoid,
                    ).then_inc(act_sem, 1)
                    # The sequencer dispatches the DMA before the ACTIVATE's
                    # writes land; gate on its completion explicitly.
                    scalar.wait_ge(act_sem, t + 1)
                    scalar.dma_start(
                        out=_dram_tile(yf, t), in_=qts[t][:]
                    ).then_inc(dma_out, 16)
                scalar.wait_ge(dma_out, 16 * NT)

    return nc


def _run(x: np.ndarray, trace: bool = False):
    x = np.ascontiguousarray(np.asarray(x), dtype=np.float16)
    assert x.shape == (B_TOTAL, N_NODES), x.shape
    nc = _build()
    in_maps = [
        {"x": x[i * ROWS_PER_CORE : (i + 1) * ROWS_PER_CORE]} for i in range(N_CORES)
    ]
    res = run_bass_kernel_spmd(nc, in_maps, list(range(N_CORES)), trace=trace)
    out = np.concatenate(
        [res.results[i]["y"] for i in range(N_CORES)], axis=0
    ).astype(np.float32)
    return out, res


def _trn_devices_visible() -> bool:
    """True when this process' jax backend exposes the 8 NeuronCores.
    A caller that pinned jax to CPU (e.g. to run the reference) hides them;
    in that case the bass run must happen in a clean subprocess."""
    try:
        import jax

        return sum(1 for d in jax.devices() if d.platform != "cpu") >= N_CORES
    except Exception:
        return False


def _run_in_subprocess(x: np.ndarray) -> np.ndarray:
    with tempfile.TemporaryDirectory() as td:
        xin = os.path.join(td, "x.npy")
        xout = os.path.join(td, "y.npy")
        np.save(xin, x)
        env = dict(os.environ)
        for k in ("JAX_PLATFORMS", "JAX_PLATFORM_NAME"):
            env.pop(k, None)
        subprocess.run(
            [sys.executable, os.path.abspath(__file__), xin, xout],
            check=True,
            env=env,
        )
        return np.load(xout)


def kernel(x, children=None, child_mask=None, parents=None, parent_mask=None,
           topo=None, **_unused):
    x = np.ascontiguousarray(np.asarray(x), dtype=np.float32)
    if _trn_devices_visible():
        out, _ = _run(x)
        return out
    return _run_in_subprocess(x)


if __name__ == "__main__":
    _x = np.load(sys.argv[1])
    _out, _ = _run(_x)
    np.save(sys.argv[2], _out)



# revision 7
# speedup vs baseline: 1.0673x; 1.0164x over previous
"""DAG-constraint layer kernel for Trainium2 (8 NeuronCores, data parallel).

The reference (p = sigmoid(x); iterative min/max projection over the
chain+skip DAG on N=32 nodes) collapses to a per-row prefix-min:

    out[b, j] = min_{k <= j} sigmoid(x[b, k]) = sigmoid(cummin(x, axis=1))

(verified bitwise against the reference by the previous session).

This version restructures the whole pipeline around measured TRN2 rates:
  - hardware scan (TensorTensorScanArith): 2.09 ns/free-elem, dtype-blind
  - tensor_tensor min, packed fp16 (2x mode): 0.52 ns/free-elem
  - ACT sigmoid: 0.833 ns/free-elem, in-place free
so the serial scan is run over only every 8th column (block minima) and
everything else is packed-fp16 elementwise work:

  1. Host: x -> fp16, rows permuted CLASS-MAJOR per partition-chunk:
     each group of G rows [G, 32] -> [G, 4, 8] -> [8(class c), G, 4(block q)]
     so column j = 8q + c of a row lands in class block c.  All device
     operands become fully contiguous packed fp16.
  2. ACT: p = sigmoid(x) in place as input chunks arrive (sigmoid is
     monotone, so it commutes with min and runs UPSTREAM of the mins,
     leaving no sigmoid tail at the drain).
  3. DVE per tile: W-chain  W_c = min(W_{c-1}, p_c)  (7 contiguous TT
     mins) gives within-block prefix minima; W_7 = per-block minimum.
  4. DVE scan over W_7 *shifted one element left* with a mask that holds
     +BIG at every block-q==0 slot: out = max(min(d0, state), d1).
     The +BIG both RESETS the running state at each row start and makes
     the scan EXCLUSIVE (S[r, q] = min over blocks < q, S[r, 0] = +BIG),
     so no repair passes are needed anywhere.
  5. DVE: m_c = min(S, W_c) (8 independent contiguous TT mins) = final
     sigmoid-domain prefix minima; DMA out; host un-permutes + casts f32.

Tolerance is 2e-2 rel; fp16 end-to-end lands ~3e-3.

Schedule: input pair-chunks tile0 on the sync DMA ring, tile1 on the
gpsimd (SWDGE) ring; sigmoid chunks on ACT gate the DVE W-chain via a
counting semaphore; output half-tiles go back on sync (first half) and
gpsimd (second half) rings.  Per-DMA semaphores give exact completion
(one DMA per semaphore, 16 increments each).
"""

import os
import subprocess
import sys
import tempfile
from contextlib import ExitStack

import numpy as np

import concourse.bass as bass
import concourse.mybir as mybir
from concourse.bass_utils import run_bass_kernel_spmd

N_CORES = 8
B_TOTAL = 524288
N_NODES = 32
ROWS_PER_CORE = B_TOTAL // N_CORES  # 65536
P = 128                             # SBUF partitions
BLK = 8                             # columns per scan block (class count)
NB = N_NODES // BLK                 # blocks per row = 4
FSIZES = [8192, 8192]               # free elems/partition per tile
NT = len(FSIZES)
POS_BIG = 60000.0                   # fp16-representable poison
NEG_BIG = -60000.0

assert sum(FSIZES) * P == ROWS_PER_CORE * N_NODES
assert all(f % N_NODES == 0 for f in FSIZES)
FC = [f // BLK for f in FSIZES]     # class-block elems (= G * NB)


def _build() -> bass.Bass:
    nc = bass.Bass()
    f16 = mybir.dt.float16
    mn = mybir.AluOpType.min
    mx = mybir.AluOpType.max
    x = nc.declare_dram_parameter("x", [ROWS_PER_CORE, N_NODES], f16, isOutput=False)
    y = nc.declare_dram_parameter("y", [ROWS_PER_CORE, N_NODES], f16, isOutput=True)
    xf = x[:].flatten()
    yf = y[:].flatten()
    offs = [0]
    for fsz in FSIZES:
        offs.append(offs[-1] + P * fsz)

    def _dram(flat, t):
        return flat[offs[t] : offs[t + 1]].rearrange("(p f) -> p f", p=P)

    with ExitStack() as es:
        ec = es.enter_context
        xts = [ec(nc.sbuf_tensor(f"xt{t}", [P, FSIZES[t]], f16)) for t in range(NT)]
        wts = [ec(nc.sbuf_tensor(f"wt{t}", [P, 7 * FC[t]], f16)) for t in range(NT)]
        sts = [ec(nc.sbuf_tensor(f"st{t}", [P, FC[t]], f16)) for t in range(NT)]
        mts = [ec(nc.sbuf_tensor(f"mt{t}", [P, FSIZES[t]], f16)) for t in range(NT)]
        mask = ec(nc.sbuf_tensor("mask", [P, max(FC)], f16))
        warm = ec(nc.sbuf_tensor("act_warm", [P, 1], f16))
        # one semaphore per input DMA: count 16 == arrived
        in_s = [ec(nc.semaphore(f"in{k}")) for k in range(4 * NT)]
        sig = ec(nc.semaphore("sig"))      # sigmoid chunks done (ACT, serial)
        msem = ec(nc.semaphore("msem"))    # m half-tiles done (DVE, serial)
        osy = ec(nc.semaphore("osy"))      # output DMAs, sync ring
        ogp = ec(nc.semaphore("ogp"))      # output DMAs, gpsimd ring

        with nc.Block() as block:

            @block.sync
            def _(sync):
                # tile 0 input, 4 class-pair chunks on the sync ring
                for k in range(4):
                    fc = FC[0]
                    sync.dma_start(
                        out=xts[0][:, 2 * k * fc : 2 * (k + 1) * fc],
                        in_=_dram(xf, 0)[:, 2 * k * fc : 2 * (k + 1) * fc],
                    ).then_inc(in_s[k], 16)
                # first half-tile outputs (classes 0-3) on the sync ring
                for t in range(NT):
                    sync.wait_ge(msem, 2 * t + 1)
                    sync.dma_start(
                        out=_dram(yf, t)[:, : 4 * FC[t]],
                        in_=mts[t][:, : 4 * FC[t]],
                    ).then_inc(osy, 16)
                sync.wait_ge(osy, 16 * NT)

            @block.gpsimd
            def _(gp):
                # tile 1 input on the SWDGE ring
                for k in range(4):
                    fc = FC[1]
                    gp.dma_start(
                        out=xts[1][:, 2 * k * fc : 2 * (k + 1) * fc],
                        in_=_dram(xf, 1)[:, 2 * k * fc : 2 * (k + 1) * fc],
                    ).then_inc(in_s[4 + k], 16)
                # second half-tile outputs (classes 4-7)
                for t in range(NT):
                    gp.wait_ge(msem, 2 * t + 2)
                    gp.dma_start(
                        out=_dram(yf, t)[:, 4 * FC[t] :],
                        in_=mts[t][:, 4 * FC[t] :],
                    ).then_inc(ogp, 16)
                gp.wait_ge(ogp, 16 * NT)

            @block.scalar
            def _(scalar):
                # dummy: pull the sigmoid table load off the critical path
                scalar.activation(
                    out=warm[:], in_=warm[:],
                    func=mybir.ActivationFunctionType.Sigmoid,
                )
                # p = sigmoid(x) in place, one chunk per input DMA
                for k in range(4 * NT):
                    t, c = divmod(k, 4)
                    fc = FC[t]
                    scalar.wait_ge(in_s[k], 16)
                    scalar.activation(
                        out=xts[t][:, 2 * c * fc : 2 * (c + 1) * fc],
                        in_=xts[t][:, 2 * c * fc : 2 * (c + 1) * fc],
                        func=mybir.ActivationFunctionType.Sigmoid,
                    ).then_inc(sig, 1)

            @block.vector
            def _(vector):
                # mask: -BIG everywhere, +BIG at block-q==0 slots (period NB)
                vector.memset(mask[:], NEG_BIG)
                vector.memset(
                    mask[:].rearrange("p (g q) -> p g q", q=NB)[:, :, 0], POS_BIG
                )
                for t in range(NT):
                    fc = FSIZES[t] // BLK
                    xt, wt, st, mt = xts[t], wts[t], sts[t], mts[t]

                    def cls(c):
                        return xt[:, c * fc : (c + 1) * fc]

                    def wc(c):  # W_c lives at [(c-1)*fc, c*fc)
                        return wt[:, (c - 1) * fc : c * fc]

                    # W-chain: within-block prefix minima (sigmoid domain)
                    vector.wait_ge(sig, 4 * t + 1)
                    vector.tensor_tensor(out=wc(1), in0=cls(0), in1=cls(1), op=mn)
                    vector.wait_ge(sig, 4 * t + 2)
                    vector.tensor_tensor(out=wc(2), in0=wc(1), in1=cls(2), op=mn)
                    vector.tensor_tensor(out=wc(3), in0=wc(2), in1=cls(3), op=mn)
                    vector.wait_ge(sig, 4 * t + 3)
                    vector.tensor_tensor(out=wc(4), in0=wc(3), in1=cls(4), op=mn)
                    vector.tensor_tensor(out=wc(5), in0=wc(4), in1=cls(5), op=mn)
                    vector.wait_ge(sig, 4 * t + 4)
                    vector.tensor_tensor(out=wc(6), in0=wc(5), in1=cls(6), op=mn)
                    vector.tensor_tensor(out=wc(7), in0=wc(6), in1=cls(7), op=mn)
                    # Exclusive segmented scan over block minima W_7:
                    # d0 = W_7 shifted one slot left (reads W_6's last elem at
                    # step 0 - poisoned), d1 = mask (+BIG at every row start:
                    # resets state AND emits the exclusive identity).
                    vector.tensor_tensor_scan(
                        out=st[:],
                        data0=wt[:, 6 * fc - 1 : 7 * fc - 1],
                        data1=mask[:, :fc],
                        initial=POS_BIG,
                        op0=mn,
                        op1=mx,
                    )
                    # m_c = min(S, W_c); independent contiguous ops
                    vector.tensor_tensor(
                        out=mt[:, :fc], in0=st[:], in1=cls(0), op=mn
                    )
                    for c in range(1, BLK):
                        op = vector.tensor_tensor(
                            out=mt[:, c * fc : (c + 1) * fc],
                            in0=st[:],
                            in1=wc(c),
                            op=mn,
                        )
                        if c in (3, BLK - 1):
                            op.then_inc(msem, 1)

    return nc


def _permute_in(x8: np.ndarray) -> np.ndarray:
    """[8, ROWS, 32] fp16 -> class-major flat [8, ROWS*32]."""
    parts = []
    row0 = 0
    for fsz in FSIZES:
        g = fsz // N_NODES
        band = x8[:, row0 : row0 + P * g].reshape(N_CORES, P, g, NB, BLK)
        parts.append(band.transpose(0, 1, 4, 2, 3).reshape(N_CORES, P * fsz))
        row0 += P * g
    return np.concatenate(parts, axis=1)


def _unpermute_out(yp: np.ndarray) -> np.ndarray:
    """[8, ROWS*32] class-major flat -> [8, ROWS, 32]."""
    outs = []
    col0 = 0
    for fsz in FSIZES:
        g = fsz // N_NODES
        band = yp[:, col0 : col0 + P * fsz].reshape(N_CORES, P, BLK, g, NB)
        outs.append(band.transpose(0, 1, 3, 4, 2).reshape(N_CORES, P * g, N_NODES))
        col0 += P * fsz
    return np.concatenate(outs, axis=1)


def _run(x: np.ndarray, trace: bool = False):
    x = np.ascontiguousarray(np.asarray(x), dtype=np.float16)
    assert x.shape == (B_TOTAL, N_NODES), x.shape
    nc = _build()
    xp = _permute_in(x.reshape(N_CORES, ROWS_PER_CORE, N_NODES))
    in_maps = [
        {"x": xp[i].reshape(ROWS_PER_CORE, N_NODES)} for i in range(N_CORES)
    ]
    res = run_bass_kernel_spmd(nc, in_maps, list(range(N_CORES)), trace=trace)
    yp = np.stack(
        [res.results[i]["y"].reshape(-1) for i in range(N_CORES)], axis=0
    )
    out = _unpermute_out(yp).reshape(B_TOTAL, N_NODES).astype(np.float32)
    return out, res


def _trn_devices_visible() -> bool:
    try:
        import jax

        return sum(1 for d in jax.devices() if d.platform != "cpu") >= N_CORES
    except Exception:
        return False


def _run_in_subprocess(x: np.ndarray) -> np.ndarray:
    with tempfile.TemporaryDirectory() as td:
        xin = os.path.join(td, "x.npy")
        xout = os.path.join(td, "y.npy")
        np.save(xin, x)
        env = dict(os.environ)
        for k in ("JAX_PLATFORMS", "JAX_PLATFORM_NAME"):
            env.pop(k, None)
        subprocess.run(
            [sys.executable, os.path.abspath(__file__), xin, xout],
            check=True,
            env=env,
        )
        return np.load(xout)


def kernel(x, children=None, child_mask=None, parents=None, parent_mask=None,
           topo=None, **_unused):
    x = np.ascontiguousarray(np.asarray(x), dtype=np.float32)
    if _trn_devices_visible():
        out, _ = _run(x)
        return out
    return _run_in_subprocess(x)


if __name__ == "__main__":
    _x = np.load(sys.argv[1])
    _out, _ = _run(_x)
    np.save(sys.argv[2], _out)


# revision 8
# speedup vs baseline: 1.0784x; 1.0104x over previous
"""DAG-constraint layer kernel for Trainium2 (8 NeuronCores, data parallel).

The reference (p = sigmoid(x); iterative min/max projection over the
chain+skip DAG on N=32 nodes) collapses to a per-row prefix-min:

    out[b, j] = min_{k <= j} sigmoid(x[b, k]) = sigmoid(cummin(x, axis=1))

(verified bitwise against the reference by an earlier session).

Measured TRN2 rates drive the design (all fp16; tolerance is 2e-2 rel and
fp16 end-to-end lands ~2.4e-3):
  - hardware scan (TensorTensorScanArith): 2.09 ns/free-elem, dtype-blind
  - tensor_tensor min, packed fp16 (2x mode): 0.52 ns/free-elem
  - ACT sigmoid: 0.833 ns/free-elem + 294 ns/instr, in-place free
  - DMA rings throttle to ~190 GB/s/ring with all 8 cores running
so the serial scan runs over only every 8th column (block minima), and
everything else is packed-fp16 elementwise work:

  1. Host permutes each partition-chunk of G rows CLASS-MAJOR:
     [G, 32] -> [G, 4, 8] -> [8(class c), G, 4(block q)], so column
     j = 8q + c lands in contiguous class block c.  Every device operand
     is then a fully contiguous packed-fp16 [P, Fc] slab.
  2. W-chain  W_c = min(W_{c-1}, p_c)  (7 TT mins): within-block prefix
     minima; W_7 = block minimum.
  3. Segmented EXCLUSIVE scan over W_7 shifted one slot left, with a mask
     holding +BIG at every block-q==0 slot: out = max(min(d0, state), d1).
     The +BIG resets the running state at each row start AND emits the
     exclusive identity, so no repair passes exist anywhere.
  4. m_c = min(S, W_c) (8 independent TT mins) = final prefix minima.

Sigmoid placement is HYBRID to kill both serial tails (sigmoid commutes
with min): tile A (processed first on DVE) takes sigmoid AFTER the min
machinery - sigma(m_A) trails mid-pipeline on ACT; tile B takes sigmoid
BEFORE (sigma(x_B) runs upstream on ACT during the DVE's tile-A phase),
so tile B's outputs DMA straight out after the last DVE op with no
trailing sigmoid.  Inputs are split across all three DMA rings (sync,
gpsimd/SWDGE, scalar) so the ~190 GB/s/ring fill phase never starves the
pipeline; outputs ride sync (even chunks) and gpsimd (odd chunks).
"""

import os
import subprocess
import sys
import tempfile
from contextlib import ExitStack

import numpy as np

import concourse.bass as bass
import concourse.mybir as mybir
from concourse.bass_utils import run_bass_kernel_spmd

N_CORES = 8
B_TOTAL = 524288
N_NODES = 32
ROWS_PER_CORE = B_TOTAL // N_CORES  # 65536
P = 128
BLK = 8                             # columns per scan block (class count)
NB = N_NODES // BLK                 # blocks per row = 4
FSIZES = [8192, 8192]               # [tile A, tile B] free elems/partition
NT = len(FSIZES)
POS_BIG = 60000.0                   # fp16-representable poison
NEG_BIG = -60000.0

assert sum(FSIZES) * P == ROWS_PER_CORE * N_NODES
assert all(f % N_NODES == 0 for f in FSIZES)
FC = [f // BLK for f in FSIZES]


def _build() -> bass.Bass:
    nc = bass.Bass()
    f16 = mybir.dt.float16
    mn = mybir.AluOpType.min
    mx = mybir.AluOpType.max
    x = nc.declare_dram_parameter("x", [ROWS_PER_CORE, N_NODES], f16, isOutput=False)
    y = nc.declare_dram_parameter("y", [ROWS_PER_CORE, N_NODES], f16, isOutput=True)
    xf = x[:].flatten()
    yf = y[:].flatten()
    offs = [0]
    for fsz in FSIZES:
        offs.append(offs[-1] + P * fsz)

    def _dram(flat, t):
        return flat[offs[t] : offs[t + 1]].rearrange("(p f) -> p f", p=P)

    def _cchunk(ap_2d, t, k):  # class-pair chunk k (classes 2k, 2k+1)
        fc = FC[t]
        return ap_2d[:, 2 * k * fc : 2 * (k + 1) * fc]

    A, B = 0, 1  # tile roles: A = sigmoid-after, B = sigmoid-first

    with ExitStack() as es:
        ec = es.enter_context
        xts = [ec(nc.sbuf_tensor(f"xt{t}", [P, FSIZES[t]], f16)) for t in range(NT)]
        wts = [ec(nc.sbuf_tensor(f"wt{t}", [P, 7 * FC[t]], f16)) for t in range(NT)]
        sts = [ec(nc.sbuf_tensor(f"st{t}", [P, FC[t]], f16)) for t in range(NT)]
        mts = [ec(nc.sbuf_tensor(f"mt{t}", [P, FSIZES[t]], f16)) for t in range(NT)]
        mask = ec(nc.sbuf_tensor("mask", [P, max(FC)], f16))
        warm = ec(nc.sbuf_tensor("act_warm", [P, 1], f16))
        in_a = [ec(nc.semaphore(f"ina{k}")) for k in range(4)]
        in_b = [ec(nc.semaphore(f"inb{k}")) for k in range(4)]
        sig = ec(nc.semaphore("sig"))    # ACT: sigma_x(B) chunks 1-4, sigma_m(A) 5-8
        msem = ec(nc.semaphore("msem"))  # DVE: m quarter-chunks, A 1-4 then B 5-8
        osy = ec(nc.semaphore("osy"))
        ogp = ec(nc.semaphore("ogp"))

        with nc.Block() as block:

            @block.sync
            def _(sync):
                # tile A classes 4-7 in; then even output chunks
                sync.dma_start(
                    out=_cchunk(xts[A][:], A, 2), in_=_cchunk(_dram(xf, A), A, 2)
                ).then_inc(in_a[2], 16)
                sync.dma_start(
                    out=_cchunk(xts[A][:], A, 3), in_=_cchunk(_dram(xf, A), A, 3)
                ).then_inc(in_a[3], 16)
                # A-outs gated on sigma_m chunks (sig 5..8): evens
                for k in (0, 2):
                    sync.wait_ge(sig, 5 + k)
                    sync.dma_start(
                        out=_cchunk(_dram(yf, A), A, k), in_=_cchunk(mts[A][:], A, k)
                    ).then_inc(osy, 16)
                # B-outs gated on msem (5..8): evens
                for k in (0, 2):
                    sync.wait_ge(msem, 5 + k)
                    sync.dma_start(
                        out=_cchunk(_dram(yf, B), B, k), in_=_cchunk(mts[B][:], B, k)
                    ).then_inc(osy, 16)
                sync.wait_ge(osy, 16 * 4)

            @block.gpsimd
            def _(gp):
                # tile A classes 0-3 in; then odd output chunks
                gp.dma_start(
                    out=_cchunk(xts[A][:], A, 0), in_=_cchunk(_dram(xf, A), A, 0)
                ).then_inc(in_a[0], 16)
                gp.dma_start(
                    out=_cchunk(xts[A][:], A, 1), in_=_cchunk(_dram(xf, A), A, 1)
                ).then_inc(in_a[1], 16)
                for k in (1, 3):
                    gp.wait_ge(sig, 5 + k)
                    gp.dma_start(
                        out=_cchunk(_dram(yf, A), A, k), in_=_cchunk(mts[A][:], A, k)
                    ).then_inc(ogp, 16)
                for k in (1, 3):
                    gp.wait_ge(msem, 5 + k)
                    gp.dma_start(
                        out=_cchunk(_dram(yf, B), B, k), in_=_cchunk(mts[B][:], B, k)
                    ).then_inc(ogp, 16)
                gp.wait_ge(ogp, 16 * 4)

            @block.scalar
            def _(scalar):
                # tile B input on the scalar ring (dispatched before any
                # activation so the ring fills during the warm-up)
                for k in range(4):
                    scalar.dma_start(
                        out=_cchunk(xts[B][:], B, k), in_=_cchunk(_dram(xf, B), B, k)
                    ).then_inc(in_b[k], 16)
                scalar.activation(
                    out=warm[:], in_=warm[:],
                    func=mybir.ActivationFunctionType.Sigmoid,
                )
                # sigma_x over tile B input chunks (upstream of DVE)
                for k in range(4):
                    scalar.wait_ge(in_b[k], 16)
                    scalar.activation(
                        out=_cchunk(xts[B][:], B, k), in_=_cchunk(xts[B][:], B, k),
                        func=mybir.ActivationFunctionType.Sigmoid,
                    ).then_inc(sig, 1)
                # sigma_m over tile A m-chunks (downstream, mid-pipeline)
                for k in range(4):
                    scalar.wait_ge(msem, k + 1)
                    scalar.activation(
                        out=_cchunk(mts[A][:], A, k), in_=_cchunk(mts[A][:], A, k),
                        func=mybir.ActivationFunctionType.Sigmoid,
                    ).then_inc(sig, 1)

            @block.vector
            def _(vector):
                # mask: -BIG everywhere, +BIG at block-q==0 slots (period NB)
                vector.memset(mask[:], NEG_BIG)
                vector.memset(
                    mask[:].rearrange("p (g q) -> p g q", q=NB)[:, :, 0], POS_BIG
                )

                def tile(t, gate_sems=None, gate_sig=False):
                    fc = FC[t]
                    xt, wt, st, mt = xts[t], wts[t], sts[t], mts[t]

                    def cls(c):
                        return xt[:, c * fc : (c + 1) * fc]

                    def wc(c):
                        return wt[:, (c - 1) * fc : c * fc]

                    def gate(k):
                        if gate_sems is not None:
                            vector.wait_ge(gate_sems[k], 16)
                        if gate_sig:
                            vector.wait_ge(sig, k + 1)

                    gate(0)
                    vector.tensor_tensor(out=wc(1), in0=cls(0), in1=cls(1), op=mn)
                    gate(1)
                    vector.tensor_tensor(out=wc(2), in0=wc(1), in1=cls(2), op=mn)
                    vector.tensor_tensor(out=wc(3), in0=wc(2), in1=cls(3), op=mn)
                    gate(2)
                    vector.tensor_tensor(out=wc(4), in0=wc(3), in1=cls(4), op=mn)
                    vector.tensor_tensor(out=wc(5), in0=wc(4), in1=cls(5), op=mn)
                    gate(3)
                    vector.tensor_tensor(out=wc(6), in0=wc(5), in1=cls(6), op=mn)
                    vector.tensor_tensor(out=wc(7), in0=wc(6), in1=cls(7), op=mn)
                    # exclusive segmented scan over shifted W_7
                    vector.tensor_tensor_scan(
                        out=st[:],
                        data0=wt[:, 6 * fc - 1 : 7 * fc - 1],
                        data1=mask[:, :fc],
                        initial=POS_BIG,
                        op0=mn,
                        op1=mx,
                    )
                    # m_c = min(S, W_c); quarter-tile msem increments
                    vector.tensor_tensor(out=mt[:, :fc], in0=st[:], in1=cls(0), op=mn)
                    for c in range(1, BLK):
                        op = vector.tensor_tensor(
                            out=mt[:, c * fc : (c + 1) * fc],
                            in0=st[:], in1=wc(c), op=mn,
                        )
                        if c % 2 == 1:
                            op.then_inc(msem, 1)

                tile(A, gate_sems=in_a)   # raw input, sigmoid comes after
                tile(B, gate_sig=True)    # sigmoid'd input, outputs final

    return nc


def _permute_in(x8: np.ndarray) -> np.ndarray:
    """[8, ROWS, 32] fp16 -> class-major flat [8, ROWS*32]."""
    parts = []
    row0 = 0
    for fsz in FSIZES:
        g = fsz // N_NODES
        band = x8[:, row0 : row0 + P * g].reshape(N_CORES, P, g, NB, BLK)
        parts.append(band.transpose(0, 1, 4, 2, 3).reshape(N_CORES, P * fsz))
        row0 += P * g
    return np.concatenate(parts, axis=1)


def _unpermute_out(yp: np.ndarray) -> np.ndarray:
    """[8, ROWS*32] class-major flat -> [8, ROWS, 32]."""
    outs = []
    col0 = 0
    for fsz in FSIZES:
        g = fsz // N_NODES
        band = yp[:, col0 : col0 + P * fsz].reshape(N_CORES, P, BLK, g, NB)
        outs.append(band.transpose(0, 1, 3, 4, 2).reshape(N_CORES, P * g, N_NODES))
        col0 += P * fsz
    return np.concatenate(outs, axis=1)


def _run(x: np.ndarray, trace: bool = False):
    x = np.ascontiguousarray(np.asarray(x), dtype=np.float16)
    assert x.shape == (B_TOTAL, N_NODES), x.shape
    nc = _build()
    xp = _permute_in(x.reshape(N_CORES, ROWS_PER_CORE, N_NODES))
    in_maps = [
        {"x": xp[i].reshape(ROWS_PER_CORE, N_NODES)} for i in range(N_CORES)
    ]
    res = run_bass_kernel_spmd(nc, in_maps, list(range(N_CORES)), trace=trace)
    yp = np.stack(
        [res.results[i]["y"].reshape(-1) for i in range(N_CORES)], axis=0
    )
    out = _unpermute_out(yp).reshape(B_TOTAL, N_NODES).astype(np.float32)
    return out, res


def _trn_devices_visible() -> bool:
    try:
        import jax

        return sum(1 for d in jax.devices() if d.platform != "cpu") >= N_CORES
    except Exception:
        return False


def _run_in_subprocess(x: np.ndarray) -> np.ndarray:
    with tempfile.TemporaryDirectory() as td:
        xin = os.path.join(td, "x.npy")
        xout = os.path.join(td, "y.npy")
        np.save(xin, x)
        env = dict(os.environ)
        for k in ("JAX_PLATFORMS", "JAX_PLATFORM_NAME"):
            env.pop(k, None)
        subprocess.run(
            [sys.executable, os.path.abspath(__file__), xin, xout],
            check=True,
            env=env,
        )
        return np.load(xout)


def kernel(x, children=None, child_mask=None, parents=None, parent_mask=None,
           topo=None, **_unused):
    x = np.ascontiguousarray(np.asarray(x), dtype=np.float32)
    if _trn_devices_visible():
        out, _ = _run(x)
        return out
    return _run_in_subprocess(x)


if __name__ == "__main__":
    _x = np.load(sys.argv[1])
    _out, _ = _run(_x)
    np.save(sys.argv[2], _out)
